# revision 38
# baseline (speedup 1.0000x reference)
"""Multi-head causal self-attention (B=2, T=2048, D=2048, 16 heads, RoPE)
on 8 Trainium2 NeuronCores.

Sharding strategy
-----------------
* Phase 1+2 (QKV projection, RoPE, attention): tensor-parallel over heads —
  each core owns 2 of the 16 heads for both batch elements, reads the full
  (host-retiled, bf16) x and its slice of qkv_w, and keeps q/k in a
  transposed [head_dim, t] bf16 layout so RoPE and the score matmuls need
  no on-device transposes.
* Attention: sT[tk, tq] = k-tile^T-free @ q, exp on the scalar engine,
  probabilities bf16, attention output lands in oT[dv, tq] and is
  normalized into bf16.
* Phase 3: FOUR small AllToAlls (one per (batch, head-slot), bf16) swap the
  head-sharded oT into sequence-sharded full-head oT. Each fires as soon as
  its (batch, head) attention completes, so the first two overlap batch-1's
  entire QKV phase and the network is never on the critical path.
* Phase 4: output projection is data-parallel over rows — each core owns
  256 rows of EACH batch element and computes them with the full out_w in
  bf16 (weights prefetched on the scalar DGE ring during attention). The
  batch-0 rows are computed while the last AllToAll is still in flight.
"""

import numpy as np

B = 2
T = 2048
D = 2048
H = 16             # global heads
HD = 128           # head dim
NCORES = 8
HPC = H // NCORES  # heads per core
NKT = D // 128     # contraction tiles over the embedding dim
TCH = 512          # t-chunk (phase 1) / tq-chunk (phase 2) width
RPB = 256          # rows per core per batch element
W = HPC * HD       # per-core q/k/v feature width (256)
T_CH = T // TCH    # chunks per batch element (4)
SCALE = 1.0 / np.sqrt(HD)

_CACHE = {}


def _build_module():
    import concourse.bacc as bacc
    import concourse.mybir as mybir
    import concourse.tile as tile

    F32 = mybir.dt.float32
    F32R = mybir.dt.float32r
    BF16 = mybir.dt.bfloat16
    ADD = mybir.AluOpType.add
    MULT = mybir.AluOpType.mult
    AF = mybir.ActivationFunctionType

    nc = bacc.Bacc("TRN2", target_bir_lowering=False, debug=False,
                   num_devices=NCORES)

    # ---- I/O ----
    xr = nc.dram_tensor("xr", [B, 16, 128, 4 * TCH], BF16,
                        kind="ExternalInput")
    wq = nc.dram_tensor("wq", [128, NKT * W], BF16, kind="ExternalInput")
    wk = nc.dram_tensor("wk", [128, NKT * W], BF16, kind="ExternalInput")
    wv = nc.dram_tensor("wv", [128, NKT * W], BF16, kind="ExternalInput")
    bqk = nc.dram_tensor("bqk", [128, 2 * HPC], F32, kind="ExternalInput")
    bvb = nc.dram_tensor("bvb", [128, W], F32, kind="ExternalInput")
    wor = nc.dram_tensor("wor", [D // TCH, 128, H * TCH], BF16,
                         kind="ExternalInput")
    bob = nc.dram_tensor("bob", [128, D], F32, kind="ExternalInput")
    cosT = nc.dram_tensor("cosT", [HD, T], F32, kind="ExternalInput")
    sinT = nc.dram_tensor("sinT", [HD, T], F32, kind="ExternalInput")
    pt = nc.dram_tensor("pt", [HD, HD], F32R, kind="ExternalInput")
    maskT = nc.dram_tensor("maskT", [HD, HD], BF16, kind="ExternalInput")
    onec = nc.dram_tensor("onec", [HD, 1], F32R, kind="ExternalInput")
    oner = nc.dram_tensor("oner", [1, HD], F32R, kind="ExternalInput")
    y = nc.dram_tensor("y", [B * RPB, D], F32, kind="ExternalOutput")

    with tile.TileContext(nc) as tc:
        frees = []

        def single(shape, dtype, name, flist=frees):
            t, free = tc.tile(shape, dtype, name=name)
            flist.append(free)
            return t

        # ---- small constants resident in SBUF (DMAs deferred) ----
        pt_sb = single([HD, HD], F32R, "pt_sb")
        mask_sb = single([HD, HD], BF16, "mask_sb")
        onec_sb = single([HD, 1], F32R, "onec_sb")
        oner_sb = single([1, HD], F32R, "oner_sb")
        bqk_sb = single([128, 2 * HPC], F32, "bqk_sb")
        bvb_sb = single([128, W], F32, "bvb_sb")
        # first quarter of out_w lives below the phase-1 working set so its
        # DMA can run during batch-0 attention instead of the phase-4 rush
        wo_sb = [single([128, H * TCH], BF16, "wo0")]

        # ---- DRAM bounce buffers for the per-(batch, head) AllToAlls ----
        with tc.tile_pool(name="dram", bufs=1, space="DRAM") as dram:
            bounce_in = [[dram.tile([NCORES * HD, RPB], BF16,
                                    name=f"bin{b}{h}") for h in range(HPC)]
                         for b in range(B)]
            bounce_out = [[dram.tile([NCORES * HD, RPB], BF16,
                                     name=f"bout{b}{h}") for h in range(HPC)]
                          for b in range(B)]

            with tc.tile_pool(name="qk_ps", bufs=2, space="PSUM") as qk_ps, \
                 tc.tile_pool(name="misc_ps", bufs=1, space="PSUM") as misc_ps, \
                 tc.tile_pool(name="st_ps", bufs=3, space="PSUM") as st_ps, \
                 tc.tile_pool(name="ot_ps", bufs=2, space="PSUM") as ot_ps:

                with tc.tile_pool(name="tmp", bufs=6) as tmp_pool, \
                     tc.tile_pool(name="et", bufs=8) as et_pool, \
                     tc.tile_pool(name="nrm", bufs=4) as nrm_pool, \
                     tc.tile_pool(name="ets", bufs=2) as ets_pool, \
                     tc.tile_pool(name="oto", bufs=4) as oto_pool:

                    # q/k/v stores for BOTH batches stay resident (bf16)
                    sfrees = []
                    q_st = [[single([128, T], BF16, f"q{b}{h}", sfrees)
                             for h in range(HPC)] for b in range(B)]
                    k_st = [[single([128, T], BF16, f"k{b}{h}", sfrees)
                             for h in range(HPC)] for b in range(B)]
                    v_st = [[single([128, T], BF16, f"v{b}{h}", sfrees)
                             for h in range(HPC)] for b in range(B)]

                    # batch-0 oc tiles live below the phase-1 working set
                    # so their loads can fire as soon as A2A #1/#2 land
                    oc = [[None] * H, [None] * H]
                    for g in range(H):
                        oc[0][g] = single([128, RPB], BF16, f"oc0{g}",
                                          sfrees)

                    # RoPE tables sit above the stores so they can be freed
                    # together with the x pool + qkv weights after phase 1
                    csfrees = []
                    cos_sb = single([HD, T], F32, "cos_sb", csfrees)
                    sin_sb = single([HD, T], F32, "sin_sb", csfrees)

                    def phase1(b, xt_pool, wq_sb, wk_sb, wv_sb,
                               js=range(T_CH)):
                        """QKV projection + RoPE for batch b."""
                        for j in js:
                            tr = slice(j * TCH, (j + 1) * TCH)
                            xq = []
                            for kq in range(4):
                                xt = xt_pool.tile([128, 4 * TCH], BF16,
                                                  name=f"x{b}{j}{kq}",
                                                  tag="xt")
                                # first chunk: x quarters 0/1 on the sync
                                # ring, 2/3 on the scalar ring behind the
                                # q-weights, so the fill is two-ring wide
                                eng = (nc.scalar if b == 0 and j == 0
                                       and kq >= 2 else nc.sync)
                                eng.dma_start(xt[:], xr.ap()[b, j * 4 + kq])
                                xq.append(xt)
                                if b == 0 and j == 0 and kq < 2:
                                    for kq2 in (2 * kq, 2 * kq + 1):
                                        qs = slice(kq2 * 4 * W,
                                                   (kq2 + 1) * 4 * W)
                                        nc.scalar.dma_start(wq_sb[:, qs],
                                                            wq.ap()[:, qs])
                            if b == 0 and j == 0:
                                nc.sync.dma_start(bqk_sb[:], bqk.ap()[:, :])
                                nc.sync.dma_start(pt_sb[:], pt.ap()[:, :])
                                nc.sync.dma_start(cos_sb[:, 0:TCH],
                                                  cosT.ap()[:, 0:TCH])
                                nc.sync.dma_start(sin_sb[:, 0:TCH],
                                                  sinT.ap()[:, 0:TCH])
                                nc.sync.dma_start(wk_sb[:], wk.ap()[:, :])
                                nc.sync.dma_start(wv_sb[:], wv.ap()[:, :])
                                nc.sync.dma_start(cos_sb[:, TCH:],
                                                  cosT.ap()[:, TCH:])
                                nc.sync.dma_start(sin_sb[:, TCH:],
                                                  sinT.ap()[:, TCH:])
                                nc.sync.dma_start(mask_sb[:],
                                                  maskT.ap()[:, :])
                                nc.sync.dma_start(onec_sb[:], onec.ap()[:, :])
                                nc.sync.dma_start(oner_sb[:], oner.ap()[:, :])
                                nc.sync.dma_start(bvb_sb[:], bvb.ap()[:, :])

                            def xsl(kt, lo, hi):
                                c0 = (kt % 4) * TCH
                                return xq[kt // 4][:, c0 + lo:c0 + hi]

                            # -- six PSUM groups: qh0 qh1 kh0 kh1 v0 v1, with
                            # the dependent rotate-matmuls trailing one group
                            # behind so the PE never head-of-line stalls.
                            rot_q = []

                            def proj_group(w_sb, h, bcol, store):
                                ps = qk_ps.tile([128, TCH], F32,
                                                name=f"ps{b}{j}{bcol}{h}",
                                                tag="qk")
                                for kt in range(NKT):
                                    col = kt * W + h * HD
                                    nc.tensor.matmul(
                                        ps[:], w_sb[:, col:col + HD],
                                        xsl(kt, 0, TCH),
                                        start=(kt == 0),
                                        stop=(kt == NKT - 1))
                                qtmp = tmp_pool.tile(
                                    [128, TCH], F32R,
                                    name=f"qt{b}{j}{bcol}{h}", tag="tmp")
                                nc.scalar.activation(
                                    qtmp[:], ps[:], AF.Identity,
                                    bias=bqk_sb[:, bcol:bcol + 1], scale=1.0)
                                rot_q.append((qtmp, store))

                            def rotate_flush():
                                qtmp, store = rot_q.pop(0)
                                rp = misc_ps.tile([128, TCH], F32,
                                                  name=f"rp{id(qtmp)}",
                                                  tag="misc")
                                nc.tensor.matmul(rp[:], pt_sb[:], qtmp[:],
                                                 start=True, stop=True)
                                t1 = tmp_pool.tile([128, TCH], F32,
                                                   name=f"t1{id(qtmp)}",
                                                   tag="tmp")
                                nc.vector.tensor_tensor(
                                    t1[:], qtmp[:], cos_sb[:, tr], MULT)
                                t2 = tmp_pool.tile([128, TCH], F32,
                                                   name=f"t2{id(qtmp)}",
                                                   tag="tmp")
                                nc.vector.tensor_tensor(
                                    t2[:], rp[:], sin_sb[:, tr], MULT)
                                nc.vector.tensor_tensor(
                                    store[:, tr], t1[:], t2[:], ADD)

                            proj_group(wq_sb, 0, 0, q_st[b][0])
                            proj_group(wq_sb, 1, 1, q_st[b][1])
                            rotate_flush()
                            proj_group(wk_sb, 0, 2, k_st[b][0])
                            rotate_flush()
                            proj_group(wk_sb, 1, 3, k_st[b][1])
                            rotate_flush()

                            # v in natural [t, dv] layout, 2 t-tiles per psum
                            for half in range(2):
                                pv = qk_ps.tile([128, TCH], F32,
                                                name=f"vps{b}{j}{half}",
                                                tag="qk")
                                for sub in range(2):
                                    ts = half * 2 + sub
                                    cs = sub * W
                                    for kt in range(NKT):
                                        nc.tensor.matmul(
                                            pv[:, cs:cs + W],
                                            xsl(kt, ts * 128, (ts + 1) * 128),
                                            wv_sb[:, kt * W:(kt + 1) * W],
                                            start=(kt == 0),
                                            stop=(kt == NKT - 1),
                                            skip_group_check=True)
                                if half == 0:
                                    rotate_flush()
                                for sub in range(2):
                                    ts = half * 2 + sub
                                    tt = j * 4 + ts
                                    for h in range(HPC):
                                        nc.vector.tensor_tensor(
                                            v_st[b][h][:, tt * 128:
                                                       (tt + 1) * 128],
                                            pv[:, sub * W + h * HD:
                                               sub * W + (h + 1) * HD],
                                            bvb_sb[:, h * HD:(h + 1) * HD],
                                            ADD)

                    def make_phase2(b, h):
                        """Chunk-granular emitter for the causal attention
                        of batch b, head-slot h, plus the AllToAll for its
                        output. The softmax normalization chain for chunk c
                        (denominator matmul, reciprocal, broadcast matmul,
                        scale, scatter) is emitted inside chunk c+1's score
                        stream so its cross-engine latency never
                        head-of-line stalls PE. Returns (chunk, finish) so
                        the caller can interleave other PE work between
                        chunks."""
                        state = {"pending": None}

                        def norm_a(p):
                            # denominator + reciprocal for a finished chunk
                            c, ot, ets = p
                            den = misc_ps.tile([1, TCH], F32,
                                               name=f"den{b}{h}{c}",
                                               tag="misc")
                            nc.tensor.matmul(den[0:1, :], onec_sb[:], ets[:],
                                             start=True, stop=True,
                                             skip_group_check=True)
                            rc = nrm_pool.tile([1, TCH], F32,
                                               name=f"rc{b}{h}{c}", tag="rc")
                            rscr = nrm_pool.tile([1, TCH], F32,
                                                 name=f"rs{b}{h}{c}",
                                                 tag="rc")
                            nc.vector.reciprocal_approx_accurate(
                                rc[:], den[0:1, :], rscr[:])
                            rcr = nrm_pool.tile([1, TCH], F32R,
                                                name=f"rr{b}{h}{c}",
                                                tag="rcr")
                            nc.scalar.copy(rcr[:], rc[:])
                            return (c, ot, rcr)

                        def norm_b(p):
                            # broadcast via rank-1 ones-matmul,
                            # normalize to bf16, scatter
                            c, ot, rcr = p
                            bc = misc_ps.tile([128, TCH], F32,
                                              name=f"bc{b}{h}{c}",
                                              tag="misc")
                            nc.tensor.matmul(bc[:], oner_sb[:], rcr[:],
                                             start=True, stop=True,
                                             skip_group_check=True)
                            bcs = nrm_pool.tile([128, TCH], F32,
                                                name=f"bcs{b}{h}{c}",
                                                tag="bcs")
                            nc.scalar.copy(bcs[:], bc[:])
                            otn = oto_pool.tile([128, TCH], BF16,
                                                name=f"otn{b}{h}{c}",
                                                tag="otn")
                            nc.vector.tensor_tensor(
                                otn[:], ot[:], bcs[:], MULT)
                            for s in range(2):
                                r = 2 * c + s
                                nc.sync.dma_start(
                                    bounce_in[b][h][r * HD:(r + 1) * HD, :],
                                    otn[:, s * RPB:(s + 1) * RPB])

                        def chunk(c):
                            ot = ot_ps.tile([128, TCH], F32,
                                            name=f"ot{b}{h}{c}", tag="ot")
                            ets = ets_pool.tile([128, TCH], F32R,
                                                name=f"ets{b}{h}{c}",
                                                tag="ets")
                            kmax = 4 * c + 3
                            q0 = c * TCH

                            ek = {}
                            issued = [0]

                            def score():
                                k = issued[0]
                                if k > kmax:
                                    return
                                issued[0] += 1
                                off = max(0, (k - 4 * c) * 128)
                                ksl = slice(k * 128, (k + 1) * 128)
                                st = st_ps.tile([128, TCH], F32,
                                                name=f"st{b}{h}{c}{k}",
                                                tag="st")
                                nc.tensor.matmul(
                                    st[:, off:TCH],
                                    k_st[b][h][:, ksl],
                                    q_st[b][h][:, q0 + off:q0 + TCH],
                                    start=True, stop=True,
                                    skip_group_check=True)
                                et = et_pool.tile([128, TCH], BF16,
                                                  name=f"et{b}{h}{c}{k}",
                                                  tag="et")
                                nc.scalar.activation(
                                    et[:, off:TCH], st[:, off:TCH],
                                    AF.Exp, bias=0.0, scale=float(SCALE))
                                if k >= 4 * c:
                                    nc.vector.tensor_tensor(
                                        et[:, off:off + 128],
                                        et[:, off:off + 128],
                                        mask_sb[:], MULT)
                                if k == 0:
                                    nc.vector.tensor_copy(ets[:], et[:])
                                else:
                                    nc.vector.tensor_tensor(
                                        ets[:, off:TCH], ets[:, off:TCH],
                                        et[:, off:TCH], ADD)
                                ek[k] = (et, off)

                            def av(k):
                                et, off = ek.pop(k)
                                ksl = slice(k * 128, (k + 1) * 128)
                                nc.tensor.matmul(
                                    ot[:, off:TCH],
                                    v_st[b][h][:, ksl],
                                    et[:, off:TCH],
                                    start=(k == 0), stop=(k == kmax),
                                    skip_group_check=True)

                            score()
                            score()
                            if state["pending"] is not None:
                                state["pending"] = norm_a(state["pending"])
                            score()
                            if state["pending"] is not None:
                                norm_b(state["pending"])
                                state["pending"] = None
                            score()
                            for k in range(kmax + 1):
                                av(k)
                                score()
                            state["pending"] = (c, ot, ets)

                        def finish():
                            norm_b(norm_a(state["pending"]))
                            state["pending"] = None
                            nc.gpsimd.collective_compute(
                                "AllToAll",
                                mybir.AluOpType.bypass,
                                replica_groups=[list(range(NCORES))],
                                ins=[bounce_in[b][h][:].opt()],
                                outs=[bounce_out[b][h][:].opt()],
                            )

                        return chunk, finish

                    # ---------------- schedule ----------------
                    with tc.tile_pool(name="xt", bufs=10) as xt_pool:
                        wfrees = []
                        wq_sb = single([128, NKT * W], BF16, "wq_sb", wfrees)
                        wk_sb = single([128, NKT * W], BF16, "wk_sb", wfrees)
                        wv_sb = single([128, NKT * W], BF16, "wv_sb", wfrees)

                        phase1(0, xt_pool, wq_sb, wk_sb, wv_sb)
                        # first out_w quarter streams in under batch-0
                        # attention while the sync ring is quiet
                        nc.sync.dma_start(wo_sb[0][:], wor.ap()[0])

                        # batch-0 attention with batch-1 projection chunks
                        # interleaved: the dense QKV matmul groups fill the
                        # attention pipeline's cross-engine bubbles and keep
                        # the PE clock-gate warm
                        c00, f00 = make_phase2(0, 0)
                        c01, f01 = make_phase2(0, 1)
                        p1 = lambda j: phase1(1, xt_pool, wq_sb, wk_sb,
                                              wv_sb, js=[j])
                        c00(0)
                        c00(1)
                        p1(0)
                        c00(2)
                        c00(3)
                        f00()                 # fires A2A #1
                        p1(1)
                        c01(0)
                        c01(1)
                        p1(2)
                        c01(2)
                        c01(3)
                        f01()                 # fires A2A #2
                        for g in range(H):
                            s, hl = g // HPC, g % HPC
                            nc.sync.dma_start(
                                oc[0][g][:],
                                bounce_out[0][hl][s * HD:(s + 1) * HD, :])
                        p1(3)
                        for f in reversed(wfrees):
                            f()
                    for f in reversed(csfrees):
                        f()
                    # x pool + qkv weights + rope tables freed: ~80 KiB/part
                    # hole for the phase-4 working set

                    ofrees = []
                    for g in range(H):
                        oc[1][g] = single([128, RPB], BF16,
                                          f"oc1{g}", ofrees)
                    wo_sb += [single([128, H * TCH], BF16, f"wo{fc}", ofrees)
                              for fc in range(1, D // TCH)]
                    bob_sb = single([128, D], F32, "bob_sb", ofrees)
                    for fc in range(1, D // TCH):
                        # remaining out_w quarters stream in under batch-1
                        # attention
                        nc.sync.dma_start(wo_sb[fc][:], wor.ap()[fc])
                    nc.sync.dma_start(bob_sb[:], bob.ap()[:, :])

                    def load_oc(b, eng=None):
                        # batch-1 loads ride the (idle-by-then) scalar ring
                        # so they never serialize behind batch-0's y stores
                        eng = eng or nc.sync
                        for g in range(H):
                            s, hl = g // HPC, g % HPC
                            eng.dma_start(
                                oc[b][g][:],
                                bounce_out[b][hl][s * HD:(s + 1) * HD, :])

                    def out_proj(b, os_pool):
                        for fc in range(D // TCH):
                            for rt in range(RPB // 128):
                                po = st_ps.tile([128, TCH], F32,
                                                name=f"po{b}{fc}{rt}",
                                                tag="st")
                                for g in range(H):
                                    nc.tensor.matmul(
                                        po[:],
                                        oc[b][g][:, rt * 128:(rt + 1) * 128],
                                        wo_sb[fc][:, g * TCH:(g + 1) * TCH],
                                        start=(g == 0), stop=(g == H - 1),
                                        skip_group_check=True)
                                os_t = os_pool.tile([128, TCH], F32,
                                                    name=f"os{b}{fc}{rt}",
                                                    tag="os")
                                nc.vector.tensor_tensor(
                                    os_t[:], po[:],
                                    bob_sb[:, fc * TCH:(fc + 1) * TCH], ADD)
                                nc.sync.dma_start(
                                    y.ap()[b * RPB + rt * 128:
                                           b * RPB + (rt + 1) * 128,
                                           fc * TCH:(fc + 1) * TCH],
                                    os_t[:])

                    c10, f10 = make_phase2(1, 0)
                    c11, f11 = make_phase2(1, 1)
                    for c in range(T_CH):
                        c10(c)
                    f10()                     # fires A2A #3
                    for c in range(T_CH):
                        c11(c)
                    f11()                     # fires A2A #4

                    with tc.tile_pool(name="os", bufs=4) as os_pool:
                        load_oc(1, eng=nc.scalar)
                        out_proj(0, os_pool)  # covers A2A #4 latency
                        out_proj(1, os_pool)

                    for f in reversed(ofrees):
                        f()
                    for f in reversed(sfrees):
                        f()

        for f in reversed(frees):
            f()

    nc.compile()
    return nc


def _host_inputs(x, qkv_w, qkv_b, out_w, out_b):
    """Build the per-core input maps (all host-side layout shuffling)."""
    import ml_dtypes

    f32 = np.float32
    bf16 = ml_dtypes.bfloat16

    x = np.asarray(x, dtype=f32)
    qkv_w = np.asarray(qkv_w, dtype=f32)
    qkv_b = np.asarray(qkv_b, dtype=f32)
    out_w = np.asarray(out_w, dtype=f32)
    out_b = np.asarray(out_b, dtype=f32)

    # x retiled to [b, j*4+kq, p, ktl*512 + c2] = x[b, j*512+c2, (kq*4+ktl)*128+p]
    xx = x.reshape(B, T_CH, TCH, NKT, 128)          # [b, j, c2, kt, p]
    xx = xx.transpose(0, 1, 3, 4, 2)                # [b, j, kt, p, c2]
    xx = xx.reshape(B, T_CH, 4, 4, 128, TCH)        # [b, j, kq, ktl, p, c2]
    xx = xx.transpose(0, 1, 2, 4, 3, 5)             # [b, j, kq, p, ktl, c2]
    xr = np.ascontiguousarray(
        xx.astype(bf16)).reshape(B, 16, 128, 4 * TCH)

    qkv_wT = qkv_w.T                                # [D, 3D]
    out_wT = out_w.T                                # [D, D]

    # out_w retiled to [fc, p, g*512+c2] = out_w[fc*512+c2, g*128+p], bf16
    ww = out_wT.reshape(H, 128, D // TCH, TCH)      # [g, p, fc, c2]
    ww = ww.transpose(2, 1, 0, 3)                   # [fc, p, g, c2]
    wor = np.ascontiguousarray(ww.astype(bf16)).reshape(
        D // TCH, 128, H * TCH)

    bob = np.ascontiguousarray(np.broadcast_to(out_b.reshape(1, D),
                                               (128, D)))

    half = HD // 2
    freq = (1.0 / (10000.0 ** (np.arange(half, dtype=np.float64) / half)))
    ang = freq[:, None] * np.arange(T, dtype=np.float64)[None, :]
    cos_h = np.cos(ang)
    sin_h = np.sin(ang)
    cosT = np.concatenate([cos_h, cos_h], axis=0).astype(f32)
    sinT = np.concatenate([sin_h, sin_h], axis=0).astype(f32)

    P = np.zeros((HD, HD), dtype=f32)
    P[np.arange(half), np.arange(half) + half] = -1.0
    P[np.arange(half) + half, np.arange(half)] = 1.0
    pt = np.ascontiguousarray(P.T)

    mask = np.where(np.arange(HD)[:, None] > np.arange(HD)[None, :],
                    f32(0.0), f32(1.0)).astype(bf16)
    onec = np.ones((HD, 1), dtype=f32)
    oner = np.ones((1, HD), dtype=f32)

    def retile_w(wslice):
        # [D, W] -> [p, kt*W + cc]
        wt = wslice.reshape(NKT, 128, W).transpose(1, 0, 2)
        return np.ascontiguousarray(wt.astype(bf16)).reshape(128, NKT * W)

    in_maps = []
    for c in range(NCORES):
        g0 = c * W
        wq_c = retile_w(qkv_wT[:, g0:g0 + W])
        wk_c = retile_w(qkv_wT[:, D + g0:D + g0 + W])
        wv_c = retile_w(qkv_wT[:, 2 * D + g0:2 * D + g0 + W])
        bqk_c = np.stack([qkv_b[g0 + h * HD:g0 + (h + 1) * HD]
                          for h in range(HPC)] +
                         [qkv_b[D + g0 + h * HD:D + g0 + (h + 1) * HD]
                          for h in range(HPC)], axis=1)     # [128, 4]
        bv_c = np.ascontiguousarray(np.broadcast_to(
            qkv_b[2 * D + g0:2 * D + g0 + W].reshape(1, W), (128, W)))
        in_maps.append({
            "xr": xr, "wq": wq_c, "wk": wk_c, "wv": wv_c,
            "bqk": np.ascontiguousarray(bqk_c), "bvb": bv_c,
            "wor": wor, "bob": bob, "cosT": cosT, "sinT": sinT,
            "pt": pt, "maskT": mask, "onec": onec, "oner": oner,
        })
    return in_maps


def kernel(x, qkv_w, qkv_b, out_w, out_b):
    from concourse.bass_utils import run_bass_kernel_spmd

    if "nc" not in _CACHE:
        _CACHE["nc"] = _build_module()
    nc = _CACHE["nc"]

    in_maps = _host_inputs(x, qkv_w, qkv_b, out_w, out_b)
    res = run_bass_kernel_spmd(nc, in_maps, core_ids=list(range(NCORES)))
    out = np.empty((B, T, D), dtype=np.float32)
    for c in range(NCORES):
        yc = res.results[c]["y"]
        out[0, c * RPB:(c + 1) * RPB] = yc[:RPB]
        out[1, c * RPB:(c + 1) * RPB] = yc[RPB:]
    return out


# revision 39
# speedup vs baseline: 1.0527x; 1.0527x over previous
"""Multi-head causal self-attention (B=2, T=2048, D=2048, 16 heads, RoPE)
on 8 Trainium2 NeuronCores.

Sharding strategy
-----------------
* Phase 1+2 (QKV projection, RoPE, attention): tensor-parallel over heads —
  each core owns 2 of the 16 heads for both batch elements, reads the full
  (host-retiled, bf16) x and its slice of qkv_w, and keeps q/k in a
  transposed [head_dim, t] bf16 layout so RoPE and the score matmuls need
  no on-device transposes.
* Attention: sT[tk, tq] = k-tile^T-free @ q, exp on the scalar engine,
  probabilities bf16, attention output lands in oT[dv, tq] and is
  normalized into bf16.
* Phase 3: FOUR small AllToAlls (one per (batch, head-slot), bf16) swap the
  head-sharded oT into sequence-sharded full-head oT. Each fires as soon as
  its (batch, head) attention completes, so the first two overlap batch-1's
  entire QKV phase and the network is never on the critical path.
* Phase 4: output projection is data-parallel over rows — each core owns
  256 rows of EACH batch element and computes them with the full out_w in
  bf16 (weights prefetched on the scalar DGE ring during attention). The
  batch-0 rows are computed while the last AllToAll is still in flight.
"""

import numpy as np

B = 2
T = 2048
D = 2048
H = 16             # global heads
HD = 128           # head dim
NCORES = 8
HPC = H // NCORES  # heads per core
NKT = D // 128     # contraction tiles over the embedding dim
TCH = 512          # t-chunk (phase 1) / tq-chunk (phase 2) width
RPB = 256          # rows per core per batch element
W = HPC * HD       # per-core q/k/v feature width (256)
T_CH = T // TCH    # chunks per batch element (4)
SCALE = 1.0 / np.sqrt(HD)

_CACHE = {}


def _build_module():
    import concourse.bacc as bacc
    import concourse.mybir as mybir
    import concourse.tile as tile

    F32 = mybir.dt.float32
    F32R = mybir.dt.float32r
    BF16 = mybir.dt.bfloat16
    ADD = mybir.AluOpType.add
    MULT = mybir.AluOpType.mult
    AF = mybir.ActivationFunctionType

    nc = bacc.Bacc("TRN2", target_bir_lowering=False, debug=False,
                   num_devices=NCORES)

    # ---- I/O ----
    xr = nc.dram_tensor("xr", [B, 16, 128, 4 * TCH], BF16,
                        kind="ExternalInput")
    wq = nc.dram_tensor("wq", [128, NKT * W], BF16, kind="ExternalInput")
    wk = nc.dram_tensor("wk", [128, NKT * W], BF16, kind="ExternalInput")
    wv = nc.dram_tensor("wv", [128, NKT * W], BF16, kind="ExternalInput")
    bqk = nc.dram_tensor("bqk", [128, 2 * HPC], F32, kind="ExternalInput")
    bvb = nc.dram_tensor("bvb", [128, W], F32, kind="ExternalInput")
    wor = nc.dram_tensor("wor", [D // TCH, 128, H * TCH], BF16,
                         kind="ExternalInput")
    bob = nc.dram_tensor("bob", [128, D], F32, kind="ExternalInput")
    cosT = nc.dram_tensor("cosT", [HD, T], BF16, kind="ExternalInput")
    sinT = nc.dram_tensor("sinT", [HD, T], BF16, kind="ExternalInput")
    pt = nc.dram_tensor("pt", [HD, HD], F32R, kind="ExternalInput")
    maskT = nc.dram_tensor("maskT", [HD, HD], BF16, kind="ExternalInput")
    onec = nc.dram_tensor("onec", [HD, 1], F32R, kind="ExternalInput")
    oner = nc.dram_tensor("oner", [1, HD], F32R, kind="ExternalInput")
    y = nc.dram_tensor("y", [B * RPB, D], F32, kind="ExternalOutput")

    with tile.TileContext(nc) as tc:
        frees = []

        def single(shape, dtype, name, flist=frees):
            t, free = tc.tile(shape, dtype, name=name)
            flist.append(free)
            return t

        # ---- small constants resident in SBUF (DMAs deferred) ----
        pt_sb = single([HD, HD], F32R, "pt_sb")
        mask_sb = single([HD, HD], BF16, "mask_sb")
        onec_sb = single([HD, 1], F32R, "onec_sb")
        oner_sb = single([1, HD], F32R, "oner_sb")
        bqk_sb = single([128, 2 * HPC], F32, "bqk_sb")
        bvb_sb = single([128, W], F32, "bvb_sb")
        # first quarter of out_w lives below the phase-1 working set so its
        # DMA can run during batch-0 attention instead of the phase-4 rush
        wo_sb = [single([128, H * TCH], BF16, "wo0")]

        # ---- DRAM bounce buffers for the per-(batch, head) AllToAlls ----
        with tc.tile_pool(name="dram", bufs=1, space="DRAM") as dram:
            bounce_in = [[dram.tile([NCORES * HD, RPB], BF16,
                                    name=f"bin{b}{h}") for h in range(HPC)]
                         for b in range(B)]
            bounce_out = [[dram.tile([NCORES * HD, RPB], BF16,
                                     name=f"bout{b}{h}") for h in range(HPC)]
                          for b in range(B)]

            with tc.tile_pool(name="qk_ps", bufs=2, space="PSUM") as qk_ps, \
                 tc.tile_pool(name="misc_ps", bufs=1, space="PSUM") as misc_ps, \
                 tc.tile_pool(name="st_ps", bufs=3, space="PSUM") as st_ps, \
                 tc.tile_pool(name="ot_ps", bufs=2, space="PSUM") as ot_ps:

                with tc.tile_pool(name="tmp", bufs=6) as tmp_pool, \
                     tc.tile_pool(name="et", bufs=10) as et_pool, \
                     tc.tile_pool(name="nrm", bufs=4) as nrm_pool, \
                     tc.tile_pool(name="ets", bufs=2) as ets_pool, \
                     tc.tile_pool(name="oto", bufs=5) as oto_pool:

                    # q/k/v stores for BOTH batches stay resident (bf16)
                    sfrees = []
                    q_st = [[single([128, T], BF16, f"q{b}{h}", sfrees)
                             for h in range(HPC)] for b in range(B)]
                    k_st = [[single([128, T], BF16, f"k{b}{h}", sfrees)
                             for h in range(HPC)] for b in range(B)]
                    v_st = [[single([128, T], BF16, f"v{b}{h}", sfrees)
                             for h in range(HPC)] for b in range(B)]

                    # batch-0 oc tiles live below the phase-1 working set
                    # so their loads can fire as soon as A2A #1/#2 land
                    oc = [[None] * H, [None] * H]
                    for g in range(H):
                        oc[0][g] = single([128, RPB], BF16, f"oc0{g}",
                                          sfrees)

                    # RoPE tables sit above the stores so they can be freed
                    # together with the x pool + qkv weights after phase 1
                    csfrees = []
                    cos_sb = single([HD, T], BF16, "cos_sb", csfrees)
                    sin_sb = single([HD, T], BF16, "sin_sb", csfrees)

                    def phase1(b, xt_pool, wq_sb, wk_sb, wv_sb,
                               js=range(T_CH)):
                        """QKV projection + RoPE for batch b."""
                        for j in js:
                            tr = slice(j * TCH, (j + 1) * TCH)
                            xq = []
                            for kq in range(4):
                                xt = xt_pool.tile([128, 4 * TCH], BF16,
                                                  name=f"x{b}{j}{kq}",
                                                  tag="xt")
                                # first chunk: x quarters 0/1 on the sync
                                # ring, 2/3 on the scalar ring behind the
                                # q-weights, so the fill is two-ring wide
                                eng = (nc.scalar if b == 0 and j == 0
                                       and kq >= 2 else nc.sync)
                                eng.dma_start(xt[:], xr.ap()[b, j * 4 + kq])
                                xq.append(xt)
                                if b == 0 and j == 0 and kq < 2:
                                    for kq2 in (2 * kq, 2 * kq + 1):
                                        qs = slice(kq2 * 4 * W,
                                                   (kq2 + 1) * 4 * W)
                                        nc.scalar.dma_start(wq_sb[:, qs],
                                                            wq.ap()[:, qs])
                            if b == 0 and j == 0:
                                nc.sync.dma_start(bqk_sb[:], bqk.ap()[:, :])
                                nc.sync.dma_start(pt_sb[:], pt.ap()[:, :])
                                nc.sync.dma_start(cos_sb[:, 0:TCH],
                                                  cosT.ap()[:, 0:TCH])
                                nc.sync.dma_start(sin_sb[:, 0:TCH],
                                                  sinT.ap()[:, 0:TCH])
                                nc.sync.dma_start(wk_sb[:], wk.ap()[:, :])
                                nc.sync.dma_start(wv_sb[:], wv.ap()[:, :])
                                nc.sync.dma_start(cos_sb[:, TCH:],
                                                  cosT.ap()[:, TCH:])
                                nc.sync.dma_start(sin_sb[:, TCH:],
                                                  sinT.ap()[:, TCH:])
                                nc.sync.dma_start(mask_sb[:],
                                                  maskT.ap()[:, :])
                                nc.sync.dma_start(onec_sb[:], onec.ap()[:, :])
                                nc.sync.dma_start(oner_sb[:], oner.ap()[:, :])
                                nc.sync.dma_start(bvb_sb[:], bvb.ap()[:, :])

                            def xsl(kt, lo, hi):
                                c0 = (kt % 4) * TCH
                                return xq[kt // 4][:, c0 + lo:c0 + hi]

                            # -- six PSUM groups: qh0 qh1 kh0 kh1 v0 v1, with
                            # the dependent rotate-matmuls trailing one group
                            # behind so the PE never head-of-line stalls.
                            rot_q = []

                            def proj_group(w_sb, h, bcol, store):
                                ps = qk_ps.tile([128, TCH], F32,
                                                name=f"ps{b}{j}{bcol}{h}",
                                                tag="qk")
                                for kt in range(NKT):
                                    col = kt * W + h * HD
                                    nc.tensor.matmul(
                                        ps[:], w_sb[:, col:col + HD],
                                        xsl(kt, 0, TCH),
                                        start=(kt == 0),
                                        stop=(kt == NKT - 1))
                                qtmp = tmp_pool.tile(
                                    [128, TCH], F32R,
                                    name=f"qt{b}{j}{bcol}{h}", tag="tmp")
                                nc.scalar.activation(
                                    qtmp[:], ps[:], AF.Identity,
                                    bias=bqk_sb[:, bcol:bcol + 1], scale=1.0)
                                rot_q.append((qtmp, store))

                            def rotate_flush():
                                qtmp, store = rot_q.pop(0)
                                rp = misc_ps.tile([128, TCH], F32,
                                                  name=f"rp{id(qtmp)}",
                                                  tag="misc")
                                nc.tensor.matmul(rp[:], pt_sb[:], qtmp[:],
                                                 start=True, stop=True)
                                t1 = tmp_pool.tile([128, TCH], F32,
                                                   name=f"t1{id(qtmp)}",
                                                   tag="tmp")
                                nc.vector.tensor_tensor(
                                    t1[:], qtmp[:], cos_sb[:, tr], MULT)
                                t2 = tmp_pool.tile([128, TCH], F32,
                                                   name=f"t2{id(qtmp)}",
                                                   tag="tmp")
                                nc.vector.tensor_tensor(
                                    t2[:], rp[:], sin_sb[:, tr], MULT)
                                nc.vector.tensor_tensor(
                                    store[:, tr], t1[:], t2[:], ADD)

                            proj_group(wq_sb, 0, 0, q_st[b][0])
                            proj_group(wq_sb, 1, 1, q_st[b][1])
                            rotate_flush()
                            proj_group(wk_sb, 0, 2, k_st[b][0])
                            rotate_flush()
                            proj_group(wk_sb, 1, 3, k_st[b][1])
                            rotate_flush()

                            # v in natural [t, dv] layout, 2 t-tiles per psum
                            for half in range(2):
                                pv = qk_ps.tile([128, TCH], F32,
                                                name=f"vps{b}{j}{half}",
                                                tag="qk")
                                for sub in range(2):
                                    ts = half * 2 + sub
                                    cs = sub * W
                                    for kt in range(NKT):
                                        nc.tensor.matmul(
                                            pv[:, cs:cs + W],
                                            xsl(kt, ts * 128, (ts + 1) * 128),
                                            wv_sb[:, kt * W:(kt + 1) * W],
                                            start=(kt == 0),
                                            stop=(kt == NKT - 1),
                                            skip_group_check=True)
                                if half == 0:
                                    rotate_flush()
                                for sub in range(2):
                                    ts = half * 2 + sub
                                    tt = j * 4 + ts
                                    for h in range(HPC):
                                        nc.vector.tensor_tensor(
                                            v_st[b][h][:, tt * 128:
                                                       (tt + 1) * 128],
                                            pv[:, sub * W + h * HD:
                                               sub * W + (h + 1) * HD],
                                            bvb_sb[:, h * HD:(h + 1) * HD],
                                            ADD)

                    def make_phase2(b, h):
                        """Chunk-granular emitter for the causal attention
                        of batch b, head-slot h, plus the AllToAll for its
                        output. The softmax normalization chain for chunk c
                        (denominator matmul, reciprocal, broadcast matmul,
                        scale, scatter) is emitted inside chunk c+1's score
                        stream so its cross-engine latency never
                        head-of-line stalls PE. Returns (chunk, finish) so
                        the caller can interleave other PE work between
                        chunks."""
                        state = {"pending": None}

                        def norm_a(p):
                            # denominator + reciprocal for a finished chunk
                            c, ot, ets = p
                            den = misc_ps.tile([1, TCH], F32,
                                               name=f"den{b}{h}{c}",
                                               tag="misc")
                            nc.tensor.matmul(den[0:1, :], onec_sb[:], ets[:],
                                             start=True, stop=True,
                                             skip_group_check=True)
                            rc = nrm_pool.tile([1, TCH], F32,
                                               name=f"rc{b}{h}{c}", tag="rc")
                            rscr = nrm_pool.tile([1, TCH], F32,
                                                 name=f"rs{b}{h}{c}",
                                                 tag="rc")
                            nc.vector.reciprocal_approx_accurate(
                                rc[:], den[0:1, :], rscr[:])
                            rcr = nrm_pool.tile([1, TCH], F32R,
                                                name=f"rr{b}{h}{c}",
                                                tag="rcr")
                            nc.scalar.copy(rcr[:], rc[:])
                            return (c, ot, rcr)

                        def norm_b(p):
                            # broadcast via rank-1 ones-matmul,
                            # normalize to bf16, scatter
                            c, ot, rcr = p
                            bc = misc_ps.tile([128, TCH], F32,
                                              name=f"bc{b}{h}{c}",
                                              tag="misc")
                            nc.tensor.matmul(bc[:], oner_sb[:], rcr[:],
                                             start=True, stop=True,
                                             skip_group_check=True)
                            bcs = nrm_pool.tile([128, TCH], F32,
                                                name=f"bcs{b}{h}{c}",
                                                tag="bcs")
                            nc.scalar.copy(bcs[:], bc[:])
                            otn = oto_pool.tile([128, TCH], BF16,
                                                name=f"otn{b}{h}{c}",
                                                tag="otn")
                            nc.vector.tensor_tensor(
                                otn[:], ot[:], bcs[:], MULT)
                            for s in range(2):
                                r = 2 * c + s
                                nc.sync.dma_start(
                                    bounce_in[b][h][r * HD:(r + 1) * HD, :],
                                    otn[:, s * RPB:(s + 1) * RPB])

                        def chunk(c):
                            ot = ot_ps.tile([128, TCH], F32,
                                            name=f"ot{b}{h}{c}", tag="ot")
                            ets = ets_pool.tile([128, TCH], F32R,
                                                name=f"ets{b}{h}{c}",
                                                tag="ets")
                            kmax = 4 * c + 3
                            q0 = c * TCH

                            ek = {}
                            issued = [0]

                            def score():
                                k = issued[0]
                                if k > kmax:
                                    return
                                issued[0] += 1
                                off = max(0, (k - 4 * c) * 128)
                                ksl = slice(k * 128, (k + 1) * 128)
                                st = st_ps.tile([128, TCH], F32,
                                                name=f"st{b}{h}{c}{k}",
                                                tag="st")
                                nc.tensor.matmul(
                                    st[:, off:TCH],
                                    k_st[b][h][:, ksl],
                                    q_st[b][h][:, q0 + off:q0 + TCH],
                                    start=True, stop=True,
                                    skip_group_check=True)
                                et = et_pool.tile([128, TCH], BF16,
                                                  name=f"et{b}{h}{c}{k}",
                                                  tag="et")
                                nc.scalar.activation(
                                    et[:, off:TCH], st[:, off:TCH],
                                    AF.Exp, bias=0.0, scale=float(SCALE))
                                if k >= 4 * c:
                                    nc.vector.tensor_tensor(
                                        et[:, off:off + 128],
                                        et[:, off:off + 128],
                                        mask_sb[:], MULT)
                                if k == 0:
                                    nc.vector.tensor_copy(ets[:], et[:])
                                else:
                                    nc.vector.tensor_tensor(
                                        ets[:, off:TCH], ets[:, off:TCH],
                                        et[:, off:TCH], ADD)
                                ek[k] = (et, off)

                            def av(k):
                                et, off = ek.pop(k)
                                ksl = slice(k * 128, (k + 1) * 128)
                                nc.tensor.matmul(
                                    ot[:, off:TCH],
                                    v_st[b][h][:, ksl],
                                    et[:, off:TCH],
                                    start=(k == 0), stop=(k == kmax),
                                    skip_group_check=True)

                            score()
                            score()
                            if state["pending"] is not None:
                                state["pending"] = norm_a(state["pending"])
                            score()
                            if state["pending"] is not None:
                                norm_b(state["pending"])
                                state["pending"] = None
                            score()
                            for k in range(kmax + 1):
                                av(k)
                                score()
                            state["pending"] = (c, ot, ets)

                        def finish():
                            norm_b(norm_a(state["pending"]))
                            state["pending"] = None
                            nc.gpsimd.collective_compute(
                                "AllToAll",
                                mybir.AluOpType.bypass,
                                replica_groups=[list(range(NCORES))],
                                ins=[bounce_in[b][h][:].opt()],
                                outs=[bounce_out[b][h][:].opt()],
                            )

                        return chunk, finish

                    # ---------------- schedule ----------------
                    with tc.tile_pool(name="xt", bufs=10) as xt_pool:
                        wfrees = []
                        wq_sb = single([128, NKT * W], BF16, "wq_sb", wfrees)
                        wk_sb = single([128, NKT * W], BF16, "wk_sb", wfrees)
                        wv_sb = single([128, NKT * W], BF16, "wv_sb", wfrees)

                        phase1(0, xt_pool, wq_sb, wk_sb, wv_sb)
                        # first out_w quarter streams in under batch-0
                        # attention while the sync ring is quiet
                        nc.sync.dma_start(wo_sb[0][:], wor.ap()[0])

                        # batch-0 attention with batch-1 projection chunks
                        # interleaved: the dense QKV matmul groups fill the
                        # attention pipeline's cross-engine bubbles and keep
                        # the PE clock-gate warm
                        c00, f00 = make_phase2(0, 0)
                        c01, f01 = make_phase2(0, 1)
                        p1 = lambda j: phase1(1, xt_pool, wq_sb, wk_sb,
                                              wv_sb, js=[j])
                        c00(0)
                        c00(1)
                        p1(0)
                        c00(2)
                        c00(3)
                        f00()                 # fires A2A #1
                        p1(1)
                        c01(0)
                        c01(1)
                        p1(2)
                        c01(2)
                        c01(3)
                        f01()                 # fires A2A #2
                        for g in range(H):
                            s, hl = g // HPC, g % HPC
                            nc.sync.dma_start(
                                oc[0][g][:],
                                bounce_out[0][hl][s * HD:(s + 1) * HD, :])
                        p1(3)
                        for f in reversed(wfrees):
                            f()
                    for f in reversed(csfrees):
                        f()
                    # x pool + qkv weights + rope tables freed: ~80 KiB/part
                    # hole for the phase-4 working set

                    ofrees = []
                    for g in range(H):
                        oc[1][g] = single([128, RPB], BF16,
                                          f"oc1{g}", ofrees)
                    wo_sb += [single([128, H * TCH], BF16, f"wo{fc}", ofrees)
                              for fc in range(1, D // TCH)]
                    bob_sb = single([128, D], F32, "bob_sb", ofrees)
                    for fc in range(1, D // TCH):
                        # remaining out_w quarters stream in under batch-1
                        # attention
                        nc.sync.dma_start(wo_sb[fc][:], wor.ap()[fc])
                    nc.sync.dma_start(bob_sb[:], bob.ap()[:, :])

                    def load_oc(b, eng=None):
                        # batch-1 loads ride the (idle-by-then) scalar ring
                        # so they never serialize behind batch-0's y stores
                        eng = eng or nc.sync
                        for g in range(H):
                            s, hl = g // HPC, g % HPC
                            eng.dma_start(
                                oc[b][g][:],
                                bounce_out[b][hl][s * HD:(s + 1) * HD, :])

                    def out_proj(b, os_pool):
                        for fc in range(D // TCH):
                            for rt in range(RPB // 128):
                                po = st_ps.tile([128, TCH], F32,
                                                name=f"po{b}{fc}{rt}",
                                                tag="st")
                                for g in range(H):
                                    nc.tensor.matmul(
                                        po[:],
                                        oc[b][g][:, rt * 128:(rt + 1) * 128],
                                        wo_sb[fc][:, g * TCH:(g + 1) * TCH],
                                        start=(g == 0), stop=(g == H - 1),
                                        skip_group_check=True)
                                os_t = os_pool.tile([128, TCH], F32,
                                                    name=f"os{b}{fc}{rt}",
                                                    tag="os")
                                nc.vector.tensor_tensor(
                                    os_t[:], po[:],
                                    bob_sb[:, fc * TCH:(fc + 1) * TCH], ADD)
                                nc.sync.dma_start(
                                    y.ap()[b * RPB + rt * 128:
                                           b * RPB + (rt + 1) * 128,
                                           fc * TCH:(fc + 1) * TCH],
                                    os_t[:])

                    c10, f10 = make_phase2(1, 0)
                    c11, f11 = make_phase2(1, 1)
                    for c in range(T_CH):
                        c10(c)
                    f10()                     # fires A2A #3
                    for c in range(T_CH):
                        c11(c)
                    f11()                     # fires A2A #4

                    with tc.tile_pool(name="os", bufs=4) as os_pool:
                        load_oc(1, eng=nc.scalar)
                        out_proj(0, os_pool)  # covers A2A #4 latency
                        out_proj(1, os_pool)

                    for f in reversed(ofrees):
                        f()
                    for f in reversed(sfrees):
                        f()

        for f in reversed(frees):
            f()

    nc.compile()
    return nc


def _host_inputs(x, qkv_w, qkv_b, out_w, out_b):
    """Build the per-core input maps (all host-side layout shuffling)."""
    import ml_dtypes

    f32 = np.float32
    bf16 = ml_dtypes.bfloat16

    x = np.asarray(x, dtype=f32)
    qkv_w = np.asarray(qkv_w, dtype=f32)
    qkv_b = np.asarray(qkv_b, dtype=f32)
    out_w = np.asarray(out_w, dtype=f32)
    out_b = np.asarray(out_b, dtype=f32)

    # x retiled to [b, j*4+kq, p, ktl*512 + c2] = x[b, j*512+c2, (kq*4+ktl)*128+p]
    xx = x.reshape(B, T_CH, TCH, NKT, 128)          # [b, j, c2, kt, p]
    xx = xx.transpose(0, 1, 3, 4, 2)                # [b, j, kt, p, c2]
    xx = xx.reshape(B, T_CH, 4, 4, 128, TCH)        # [b, j, kq, ktl, p, c2]
    xx = xx.transpose(0, 1, 2, 4, 3, 5)             # [b, j, kq, p, ktl, c2]
    xr = np.ascontiguousarray(
        xx.astype(bf16)).reshape(B, 16, 128, 4 * TCH)

    qkv_wT = qkv_w.T                                # [D, 3D]
    out_wT = out_w.T                                # [D, D]

    # out_w retiled to [fc, p, g*512+c2] = out_w[fc*512+c2, g*128+p], bf16
    ww = out_wT.reshape(H, 128, D // TCH, TCH)      # [g, p, fc, c2]
    ww = ww.transpose(2, 1, 0, 3)                   # [fc, p, g, c2]
    wor = np.ascontiguousarray(ww.astype(bf16)).reshape(
        D // TCH, 128, H * TCH)

    bob = np.ascontiguousarray(np.broadcast_to(out_b.reshape(1, D),
                                               (128, D)))

    half = HD // 2
    freq = (1.0 / (10000.0 ** (np.arange(half, dtype=np.float64) / half)))
    ang = freq[:, None] * np.arange(T, dtype=np.float64)[None, :]
    cos_h = np.cos(ang)
    sin_h = np.sin(ang)
    cosT = np.concatenate([cos_h, cos_h], axis=0).astype(bf16)
    sinT = np.concatenate([sin_h, sin_h], axis=0).astype(bf16)

    P = np.zeros((HD, HD), dtype=f32)
    P[np.arange(half), np.arange(half) + half] = -1.0
    P[np.arange(half) + half, np.arange(half)] = 1.0
    pt = np.ascontiguousarray(P.T)

    mask = np.where(np.arange(HD)[:, None] > np.arange(HD)[None, :],
                    f32(0.0), f32(1.0)).astype(bf16)
    onec = np.ones((HD, 1), dtype=f32)
    oner = np.ones((1, HD), dtype=f32)

    def retile_w(wslice):
        # [D, W] -> [p, kt*W + cc]
        wt = wslice.reshape(NKT, 128, W).transpose(1, 0, 2)
        return np.ascontiguousarray(wt.astype(bf16)).reshape(128, NKT * W)

    in_maps = []
    for c in range(NCORES):
        g0 = c * W
        wq_c = retile_w(qkv_wT[:, g0:g0 + W])
        wk_c = retile_w(qkv_wT[:, D + g0:D + g0 + W])
        wv_c = retile_w(qkv_wT[:, 2 * D + g0:2 * D + g0 + W])
        bqk_c = np.stack([qkv_b[g0 + h * HD:g0 + (h + 1) * HD]
                          for h in range(HPC)] +
                         [qkv_b[D + g0 + h * HD:D + g0 + (h + 1) * HD]
                          for h in range(HPC)], axis=1)     # [128, 4]
        bv_c = np.ascontiguousarray(np.broadcast_to(
            qkv_b[2 * D + g0:2 * D + g0 + W].reshape(1, W), (128, W)))
        in_maps.append({
            "xr": xr, "wq": wq_c, "wk": wk_c, "wv": wv_c,
            "bqk": np.ascontiguousarray(bqk_c), "bvb": bv_c,
            "wor": wor, "bob": bob, "cosT": cosT, "sinT": sinT,
            "pt": pt, "maskT": mask, "onec": onec, "oner": oner,
        })
    return in_maps


def kernel(x, qkv_w, qkv_b, out_w, out_b):
    from concourse.bass_utils import run_bass_kernel_spmd

    if "nc" not in _CACHE:
        _CACHE["nc"] = _build_module()
    nc = _CACHE["nc"]

    in_maps = _host_inputs(x, qkv_w, qkv_b, out_w, out_b)
    res = run_bass_kernel_spmd(nc, in_maps, core_ids=list(range(NCORES)))
    out = np.empty((B, T, D), dtype=np.float32)
    for c in range(NCORES):
        yc = res.results[c]["y"]
        out[0, c * RPB:(c + 1) * RPB] = yc[:RPB]
        out[1, c * RPB:(c + 1) * RPB] = yc[RPB:]
    return out
